# revision 1
# baseline (speedup 1.0000x reference)
"""Bass/Tile kernel for chunkwise retention (nn_ChunkwiseRetention).

Algorithm (per core = one batch element, seq 4000, B=5, 800 chunks):
superchunks of G=25 chunks (125 positions). The host pre-scales
xqT columns by g6^j and xkT by g6^-j (j = global chunk index), which
folds the entire cross-chunk decay into the projections: the cross
mask becomes 0/1, the carry is Q~ @ U with no rescale, and the state
update needs no scaling at all.

Per superchunk s: Q~^T,K~^T (dim-major, projected 4 superchunks at a
time at N=500) and K~,V (pos-major) projections; P~^T = K~ @ Q~^T;
masked matmuls accumulate cross + intra (+5-row shift via
free-dim-shifted stationary) + seam (previous superchunk's tail
stationary x previous V) + carry (Q~ @ U) into one PSUM window;
running state U in one PSUM bank (zero-matmul init, per-element
has_written accumulation).

All big matmuls run with float32r operands (full-rate fp32 on the PE at
even moving-dim >= 256; plain fp32 is 4 cycles/row). P^T (N=125, odd)
runs in plain f32 — same cost at N<256.

PSUM banks (8): qkt(shared) 2 + kv 2 + pt 1 + wt 2 + u 1.
"""
import numpy as np

import concourse.bass as bass
import concourse.mybir as mybir
import concourse.tile as tile

GAMMA = 0.9865
B = 5
SEQ = 4000
FEAT = 256
DIM = 256
G = 25
GP = G * B            # 125
NSC = SEQ // GP       # 32
LG = 4                # superchunks per projection/load group
LGP = LG * GP         # 500
F32 = mybir.dt.float32
F32R = mybir.dt.float32r
g6 = float(np.float64(GAMMA) ** 6)
COPY = mybir.ActivationFunctionType.Copy

# const blob column layout
C_WCT = 0            # [0:125)   0/1 strict lower-block-triangular cross mask
C_WIT = 125          # [125:250) intra decay mask (rows 0:125)
C_I5 = 250           # [250:375) I5 seam selector (rows 0:5)
C_Z = 375            # [375:887) zeros (row 0 used as zero matmul operand)
C_END = 887


def make_const_blob():
    t = np.arange(GP) // B
    p = np.arange(GP) % B
    tb, ta = t[:, None], t[None, :]
    wct01 = (tb < ta).astype(np.float32)
    qb, pa = p[:, None], p[None, :]
    wit = np.where((tb == ta) & (pa >= qb),
                   np.float64(GAMMA) ** (qb - pa), 0.0).astype(np.float32)
    blob = np.zeros((128, C_END), np.float32)
    blob[0:GP, C_WCT:C_WCT + 125] = wct01
    blob[0:GP, C_WIT:C_WIT + 125] = wit
    blob[0:B, C_I5:C_I5 + B] = np.eye(B, dtype=np.float32)  # I5 in cols 250:255
    return blob


def build_kernel(nc: bass.Bass):
    xqT = nc.dram_tensor("xqT", [FEAT, SEQ], F32R, kind="ExternalInput").ap()
    xkT = nc.dram_tensor("xkT", [FEAT, SEQ], F32R, kind="ExternalInput").ap()
    xvT = nc.dram_tensor("xvT", [FEAT, SEQ], F32R, kind="ExternalInput").ap()
    wqkv = nc.dram_tensor("wqkv", [FEAT, 3 * DIM], F32R, kind="ExternalInput").ap()
    out = nc.dram_tensor("out", [SEQ, DIM], F32, kind="ExternalOutput").ap()

    blob_np = make_const_blob()

    def mm(o, l, r_, **kw):
        nc.tensor.matmul(o, l.bitcast(F32R), r_.bitcast(F32R), **kw)

    with tile.TileContext(nc) as tc:
        with (
            tc.tile_pool(name="consts", bufs=1) as cpool,
            tc.tile_pool(name="xin", bufs=2) as xpool,
            tc.tile_pool(name="work", bufs=2) as spool,
            tc.tile_pool(name="psT", bufs=1, space="PSUM") as psT,
            tc.tile_pool(name="psP", bufs=2, space="PSUM") as psP,
            tc.tile_pool(name="psPT", bufs=1, space="PSUM") as psPT,
            tc.tile_pool(name="psW", bufs=2, space="PSUM") as psW,
            tc.tile_pool(name="psU", bufs=1, space="PSUM") as psU,
        ):
            # --- constants to SBUF: exactly two DMAs (blob + packed weights) ---
            blob_sb = cpool.tile([128, C_END], F32R, name="blob_sb")
            nc.sync.dma_start(out=blob_sb,
                              in_=nc.inline_tensor(blob_np, "cblob").ap().bitcast(F32R))
            wct_sb = blob_sb[0:GP, C_WCT:C_WCT + 125]
            wit_sb = blob_sb[0:GP, C_WIT:C_WIT + 125]
            i5_sb = blob_sb[0:B, C_I5:C_I5 + 125]
            w_sb = cpool.tile_from(wqkv.rearrange("(h p) d -> p h d", p=128))
            wk_sb = w_sb[:, :, 256:512]
            wv_sb = w_sb[:, :, 512:768]

            u_ps = psU.tile([128, 512], F32, name="u_state")

            # preamble: absorb the weights-DMA wait into one dummy matmul and
            # the const-blob DMA wait into one dummy DVE copy (fewer split
            # waits); zero-matmul initializes the U bank's data + has_written
            # bits so the per-superchunk state matmuls can all accumulate.
            nc.tensor.matmul(u_ps[0:1, 0:1], w_sb[:, 0, 0:1].bitcast(F32),
                             w_sb[:, 0, 0:1].bitcast(F32),
                             start=True, stop=True, skip_group_check=True)
            scratch_sb = spool.tile([1, 1], F32, name="scratch", tag="scratch")
            nc.vector.tensor_copy(scratch_sb, blob_sb[0:1, 0:1])
            mm(u_ps, blob_sb[0:1, C_Z:C_Z + 128], blob_sb[0:1, C_Z:C_Z + 512],
               start=True, stop=True, skip_group_check=True)

            # persistent mpi stationaries (manual double-buffer): zero columns
            # are memset once; the per-superchunk mul only rewrites cols 5:130
            mpi_bufs = []
            for i_ in range(3):
                mb_ = spool.tile([125, 250], F32R, name=f"mpi_{i_}", tag=f"mpi_{i_}",
                                 bufs=1)
                nc.vector.memset(mb_[:, 0:5].bitcast(F32), 0.0)
                nc.vector.memset(mb_[:, 130:250].bitcast(F32), 0.0)
                mpi_bufs.append(mb_)

            prev_mpi = prev_v = None
            xg = {}
            qkt_sb = {}

            def load_group(gidx):
                gsl = slice(gidx * LGP, (gidx + 1) * LGP)
                xq_g = xpool.tile([128, 2, LGP], F32R, name=f"xq_{gidx}", tag="xq")
                xk_g = xpool.tile([128, 2, LGP], F32R, name=f"xk_{gidx}", tag="xk")
                xv_g = xpool.tile([128, 2, LGP], F32R, name=f"xv_{gidx}", tag="xv")
                nc.sync.dma_start(out=xq_g, in_=xqT[:, gsl].rearrange("(h p) a -> p h a", p=128))
                nc.sync.dma_start(out=xk_g, in_=xkT[:, gsl].rearrange("(h p) a -> p h a", p=128))
                nc.sync.dma_start(out=xv_g, in_=xvT[:, gsl].rearrange("(h p) a -> p h a", p=128))
                xg["x"] = (xq_g, xk_g, xv_g)
                # Q~^T / K~^T projections for the group (N=500), via a shared
                # psum tag: d-lo cols 0:500 (bank 0), d-hi cols 512:1012
                # (bank 1), each bank one closed accumulation group
                qt_sb = spool.tile([128, 1000], F32R, name=f"qt_{gidx}", tag="qt")
                kt_sb = spool.tile([128, 1000], F32R, name=f"kt_{gidx}", tag="kt")
                qkt_q = psT.tile([128, 1024], F32, name=f"qkt_q_{gidx}", tag="qkt")
                for off, dlo in ((0, 0), (512, 128)):
                    for h in (0, 1):
                        mm(qkt_q[:, off:off + LGP], w_sb[:, h, dlo:dlo + 128],
                           xq_g[:, h, :], start=(h == 0), stop=(h == 1))
                nc.scalar.activation(qt_sb[:, 0:500], qkt_q[:, 0:500], COPY)
                nc.scalar.activation(qt_sb[:, 500:1000], qkt_q[:, 512:1012], COPY)
                qkt_k = psT.tile([128, 1024], F32, name=f"qkt_k_{gidx}", tag="qkt")
                for off, dlo in ((0, 256), (512, 384)):
                    for h in (0, 1):
                        mm(qkt_k[:, off:off + LGP], w_sb[:, h, dlo:dlo + 128],
                           xk_g[:, h, :], start=(h == 0), stop=(h == 1))
                nc.scalar.activation(kt_sb[:, 0:500], qkt_k[:, 0:500], COPY)
                nc.scalar.activation(kt_sb[:, 500:1000], qkt_k[:, 512:1012], COPY)
                qkt_sb["qk"] = (qt_sb, kt_sb)

            def prep_sc(s):
                """K~/V/Vw + P~^T + masked stationaries for superchunk s."""
                gidx, ls = divmod(s, LG)
                if ls == 0:
                    load_group(gidx)
                xq_g, xk_g, xv_g = xg["x"]
                qt_sb, kt_sb = qkt_sb["qk"]
                lsl = slice(ls * GP, (ls + 1) * GP)
                xk_s = xk_g[:, :, lsl]
                xv_s = xv_g[:, :, lsl]
                qlo = qt_sb[:, ls * GP:(ls + 1) * GP]
                qhi = qt_sb[:, 500 + ls * GP:500 + (ls + 1) * GP]
                klo = kt_sb[:, ls * GP:(ls + 1) * GP]
                khi = kt_sb[:, 500 + ls * GP:500 + (ls + 1) * GP]

                # K~/V pos-major: K~ cols 0:256, V cols 256:512
                kv = psP.tile([125, 512], F32, name=f"kv_{s}", tag="kv")
                for h in (0, 1):
                    mm(kv[:, 0:256], xk_s[:, h, :], wk_sb[:, h, :],
                       start=(h == 0), stop=(h == 1))
                for h in (0, 1):
                    mm(kv[:, 256:512], xv_s[:, h, :], wv_sb[:, h, :],
                       start=(h == 0), stop=(h == 1))
                kv_sb = spool.tile([125, 512], F32R, name=f"kv_sb_{s}", tag="kvsb", bufs=3)
                nc.vector.tensor_copy(kv_sb, kv)
                k_sb = kv_sb[:, 0:256]
                v_sb = kv_sb[:, 256:512]

                # P~^T = K~ @ Q~^T (N=125 odd -> plain f32; at N<256 f32r is
                # 4 cycles/row anyway, so this costs the same)
                pt_ps = psPT.tile([125, 125], F32, name=f"pt_{s}", tag="pt")
                nc.tensor.matmul(pt_ps, klo.bitcast(F32), qlo.bitcast(F32),
                                 start=True, stop=False)
                nc.tensor.matmul(pt_ps, khi.bitcast(F32), qhi.bitcast(F32),
                                 start=False, stop=True)

                mpc_sb = spool.tile([125, 125], F32R, name=f"mpc_{s}", tag="mpc", bufs=3)
                mpi_sb = mpi_bufs[s % 3]
                nc.vector.tensor_mul(mpc_sb, pt_ps, wct_sb)
                nc.vector.tensor_mul(mpi_sb[:, 5:130], pt_ps, wit_sb)
                return dict(k_sb=k_sb, v_sb=v_sb,
                            mpc_sb=mpc_sb, mpi_sb=mpi_sb, qlo=qlo, qhi=qhi)

            st = prep_sc(0)
            for s in range(NSC):
                k_sb, v_sb = st["k_sb"], st["v_sb"]
                mpc_sb, mpi_sb = st["mpc_sb"], st["mpi_sb"]
                qlo, qhi = st["qlo"], st["qhi"]

                # --- window accumulation (one closed group). The seam
                # (intra tail of chunk s*G-1) is added directly from the
                # previous superchunk's tail stationary and V: rows 5:125 of
                # that matmul multiply zero columns and accumulate zeros. ---
                wt = psW.tile([125, 256], F32, name=f"wt_{s}", tag="wt")
                mm(wt, mpc_sb, v_sb, start=True, stop=False)
                mm(wt, mpi_sb[:, 0:125], v_sb, start=False, stop=(s == 0))
                if s > 0:
                    ut_sb = spool.tile([128, 512], F32R, name=f"ut_{s}", tag="ut")
                    nc.scalar.activation(ut_sb, u_ps, COPY)
                    mm(wt, prev_mpi[:, 125:250], prev_v, start=False, stop=False)
                    mm(wt, qlo, ut_sb[:, 0:256], start=False, stop=False)
                    mm(wt, qhi, ut_sb[:, 256:512], start=False, stop=True)

                # --- state update (accumulates; U bank bits set by zero-mm) ---
                mm(u_ps[:, 0:256], k_sb[:, 0:128], v_sb,
                   start=False, stop=True, skip_group_check=True)
                mm(u_ps[:, 256:512], k_sb[:, 128:256], v_sb,
                   start=False, stop=True, skip_group_check=True)

                # pipeline: prepare s+1 so PE has projection/PT work in flight
                # while DVE produces the next masks
                if s + 1 < NSC:
                    st = prep_sc(s + 1)

                # --- output ---
                wall_sb = spool.tile([125, 256], F32, name=f"wall_{s}", tag="wall", bufs=3)
                nc.vector.tensor_copy(wall_sb, wt)
                if s == 0:
                    nc.sync.dma_start(out=out[0:GP - B], in_=wall_sb[B:GP])
                else:
                    nc.sync.dma_start(out=out[s * GP - B: s * GP - B + GP],
                                      in_=wall_sb)
                prev_mpi, prev_v = mpi_sb, v_sb

            # final output chunk 799 = intra tail of the last superchunk
            wtf = psW.tile([125, 256], F32, name="wt_final", tag="wt")
            mm(wtf, prev_mpi[:, 125:250], prev_v, start=True, stop=True)
            wallf_sb = spool.tile([5, 256], F32, name="wallf", tag="wallf")
            nc.vector.tensor_copy(wallf_sb, wtf[0:5])
            nc.sync.dma_start(out=out[SEQ - B:SEQ], in_=wallf_sb)

    return nc


def _col_scales():
    j = np.arange(SEQ) // B          # global chunk index
    sq = (np.float64(g6) ** j).astype(np.float32)
    sk = (np.float64(g6) ** (-j)).astype(np.float32)
    return sq, sk


def prep_core_inputs(xq2d, xk2d, xv2d, wqkv):
    sq, sk = _col_scales()
    return {
        "xqT": np.ascontiguousarray(xq2d.T * sq[None, :]),
        "xkT": np.ascontiguousarray(xk2d.T * sk[None, :]),
        "xvT": np.ascontiguousarray(xv2d.T),
        "wqkv": wqkv,
    }


def make_in_maps(inputs):
    """inputs: dict from setup_inputs (full batch). Returns per-core in_maps."""
    xq, xk, xv = inputs["xq"], inputs["xk"], inputs["xv"]
    wqkv = np.ascontiguousarray(np.concatenate(
        [np.asarray(inputs["Wq"], dtype=np.float32),
         np.asarray(inputs["Wk"], dtype=np.float32),
         np.asarray(inputs["Wv"], dtype=np.float32)], axis=1))
    in_maps = []
    for b in range(8):
        in_maps.append(prep_core_inputs(
            np.asarray(xq[b], dtype=np.float32),
            np.asarray(xk[b], dtype=np.float32),
            np.asarray(xv[b], dtype=np.float32), wqkv))
    return in_maps


_NC_CACHE = {}


def _get_nc():
    if "nc" not in _NC_CACHE:
        from concourse import bacc
        nc = bacc.Bacc("TRN2", target_bir_lowering=False, debug=False)
        build_kernel(nc)
        nc.compile()
        _NC_CACHE["nc"] = nc
    return _NC_CACHE["nc"]


def run(inputs, trace=False, **kwargs):
    """Run on 8 NeuronCores; returns (output [8,4000,256], BassKernelResults)."""
    from concourse.bass_utils import run_bass_kernel_spmd

    nc = _get_nc()
    in_maps = make_in_maps(inputs)
    res = run_bass_kernel_spmd(nc, in_maps, core_ids=list(range(8)),
                               trace=trace, **kwargs)
    out = np.stack([r["out"] for r in res.results], axis=0)
    return out, res


def kernel(**inputs) -> np.ndarray:
    out, _ = run(inputs)
    return out



# revision 3
# speedup vs baseline: 1.0623x; 1.0623x over previous
"""Bass/Tile kernel for chunkwise retention (nn_ChunkwiseRetention).

Shifted-window scheme (v2), per core = one batch element, seq 4000, B=5:

Windows of 125 positions shifted by -5: window s covers output positions
[s*125-5, s*125+120), and the V/K contraction range is the SAME shifted
span, so the seam (intra of the chunk straddling the superchunk boundary)
folds into the single combined masked matmul — no separate seam matmul.
The carry boundary moves one chunk earlier: carry_s = Q[s*125:+125] @
U_shift(s-1) with U_shift accumulating K^T V over shifted windows.

Host pre-scales xqT columns by g6^j and xkT by g6^-j (j = chunk index),
folding all cross-chunk decay into the projections (cross mask is 0/1).
All inputs and SBUF operands are bf16 (PE: 1 cycle/row at any moving
width, halved DMA); PSUM accumulation stays f32.

Per iteration s: V proj (s+1, shifted window), window matmuls for s
(comb + carry into one PSUM group), P~^T (s+1) = K~^T_slice x Q~^T_slice
(N=130: q cols shifted -5..+125), state update, group-ahead Q^T/K^T
projections (N=505), K pos-major via PE transpose, DVE mask build
(comb = wct*P[:,5:130] + wit*P[:,0:125]), Pool PSUM->SBUF copies.
Tail (chunk 799, intra-only) is issued early at s==27.

PSUM banks (8): qkt 2 + v 1 + pt 1 + ktr 1 + wt 2 + u 1.
"""
import numpy as np
import ml_dtypes

import concourse.bass as bass
import concourse.mybir as mybir
import concourse.tile as tile

GAMMA = 0.9865
B = 5
SEQ = 4000
FEAT = 256
DIM = 256
GP = 125              # window size (25 chunks)
NSC = SEQ // GP       # 32
NG = 8                # groups of 4 windows
GW = 505              # group buffer width (500 + 5 shift overlap)
F32 = mybir.dt.float32
F32R = mybir.dt.float32r
BF16 = mybir.dt.bfloat16
g6 = float(np.float64(GAMMA) ** 6)
COPY = mybir.ActivationFunctionType.Copy

# const blob column layout (f32)
C_WIT = 0             # [0:125)   intra mask, shifted coords
C_WCT = 125           # [125:250) 0/1 cross mask, shifted coords
C_Z = 250             # [250:762) zeros (row 0: zero matmul operands)
C_END = 762


def make_const_blob():
    j = np.arange(GP)
    jj, rr = j[:, None], j[None, :]
    witn = np.where((jj // B == rr // B) & (rr % B >= jj % B),
                    np.float64(GAMMA) ** (jj % B - rr % B), 0.0)
    wctn = (jj // B <= rr // B).astype(np.float64)
    blob = np.zeros((128, C_END), np.float32)
    blob[0:GP, C_WIT:C_WIT + GP] = witn.astype(np.float32)
    blob[0:GP, C_WCT:C_WCT + GP] = wctn.astype(np.float32)
    return blob


def build_kernel(nc: bass.Bass):
    xqT = nc.dram_tensor("xqT", [FEAT, SEQ], BF16, kind="ExternalInput").ap()
    xkT = nc.dram_tensor("xkT", [FEAT, SEQ], BF16, kind="ExternalInput").ap()
    xvT = nc.dram_tensor("xvT", [FEAT, SEQ], BF16, kind="ExternalInput").ap()
    wqkv = nc.dram_tensor("wqkv", [FEAT, 3 * DIM], BF16, kind="ExternalInput").ap()
    out = nc.dram_tensor("out", [SEQ, DIM], F32, kind="ExternalOutput").ap()

    blob_np = make_const_blob()
    ident_np = np.eye(128, dtype=ml_dtypes.bfloat16)
    mm = nc.tensor.matmul

    with tile.TileContext(nc) as tc:
        with (
            tc.tile_pool(name="consts", bufs=1) as cpool,
            tc.tile_pool(name="xin", bufs=3) as xpool,
            tc.tile_pool(name="qkt", bufs=2) as qpool,
            tc.tile_pool(name="work", bufs=2) as spool,
            tc.tile_pool(name="psT", bufs=1, space="PSUM") as psT,
            tc.tile_pool(name="psV", bufs=1, space="PSUM") as psV,
            tc.tile_pool(name="psPT", bufs=1, space="PSUM") as psPT,
            tc.tile_pool(name="psKT", bufs=1, space="PSUM") as psKT,
            tc.tile_pool(name="psW", bufs=2, space="PSUM") as psW,
            tc.tile_pool(name="psU", bufs=1, space="PSUM") as psU,
        ):
            # --- constants: blob (f32), bf16 identity, packed weights ---
            blob_sb = cpool.tile([128, C_END], F32, name="blob_sb")
            nc.sync.dma_start(out=blob_sb, in_=nc.inline_tensor(blob_np, "cblob").ap())
            ident_sb = cpool.tile([128, 128], BF16, name="ident_sb")
            nc.sync.dma_start(out=ident_sb, in_=nc.inline_tensor(ident_np, "cident").ap())
            wit_sb = blob_sb[0:GP, C_WIT:C_WIT + GP]
            wct_sb = blob_sb[0:GP, C_WCT:C_WCT + GP]
            w_sb = cpool.tile_from(wqkv.rearrange("(h p) d -> p h d", p=128))

            u_ps = psU.tile([128, 512], F32, name="u_state")

            # preamble: absorb const/weight DMA waits; zero-matmul sets the
            # U bank's data + has_written bits so state matmuls accumulate.
            mm(u_ps[0:1, 0:1], w_sb[:, 0, 0:1], w_sb[:, 0, 0:1],
               start=True, stop=True, skip_group_check=True)
            scr = spool.tile([1, 1], F32, name="scr", tag="scr")
            nc.vector.tensor_copy(scr, blob_sb[0:1, 0:1])
            scr2 = spool.tile([1, 1], BF16, name="scr2", tag="scr2")
            nc.scalar.activation(scr2, ident_sb[0:1, 0:1], COPY)
            mm(u_ps, blob_sb[0:1, C_Z:C_Z + 128].bitcast(F32R),
               blob_sb[0:1, C_Z:C_Z + 512].bitcast(F32R),
               start=True, stop=True, skip_group_check=True)

            xg = {}
            qts = {}
            kts = {}

            def load_group_x(g):
                tiles = []
                for nm, src in (("xq", xqT), ("xk", xkT), ("xv", xvT)):
                    t = xpool.tile([128, 2, GW], BF16, name=f"{nm}_{g}", tag=nm)
                    if g == 0:
                        nc.vector.memset(t[:, :, 0:5], 0.0)
                        nc.sync.dma_start(
                            out=t[:, :, 5:GW],
                            in_=src[:, 0:500].rearrange("(h p) a -> p h a", p=128))
                    else:
                        nc.sync.dma_start(
                            out=t,
                            in_=src[:, g * 500 - 5:g * 500 + 500]
                                .rearrange("(h p) a -> p h a", p=128))
                    tiles.append(t)
                xg[g] = tiles

            def proj_qkt(g, which):
                x = xg[g][0 if which == "qt" else 1]
                dlos = (0, 128) if which == "qt" else (256, 384)
                ps = psT.tile([128, 1024], F32, name=f"ps_{which}_{g}", tag="qkt")
                for off, dlo in ((0, dlos[0]), (512, dlos[1])):
                    for h in (0, 1):
                        mm(ps[:, off:off + GW], w_sb[:, h, dlo:dlo + 128],
                           x[:, h, :], start=(h == 0), stop=(h == 1))
                t = qpool.tile([128, 2 * GW], BF16, name=f"{which}_{g}", tag=which)
                nc.scalar.activation(t[:, 0:GW], ps[:, 0:GW], COPY)
                nc.scalar.activation(t[:, GW:2 * GW], ps[:, 512:512 + GW], COPY)
                (qts if which == "qt" else kts)[g] = t

            def vproj(s):
                g, ls = divmod(s, 4)
                xv = xg[g][2]
                vs = psV.tile([GP, 256], F32, name=f"v_{s}", tag="v")
                for h in (0, 1):
                    mm(vs, xv[:, h, ls * GP:ls * GP + GP], w_sb[:, h, 512:768],
                       start=(h == 0), stop=(h == 1))
                v_sb = spool.tile([GP, 256], BF16, name=f"vsb_{s}", tag="vsb")
                nc.vector.tensor_copy(v_sb, vs)
                return v_sb

            def ptmm(s):
                g, ls = divmod(s, 4)
                qt, kt = qts[g], kts[g]
                pt = psPT.tile([GP, 130], F32, name=f"pt_{s}", tag="pt")
                for h, off in ((0, 0), (1, GW)):
                    mm(pt, kt[:, off + ls * GP:off + ls * GP + GP],
                       qt[:, off + ls * GP:off + ls * GP + 130],
                       start=(h == 0), stop=(h == 1))
                return pt

            def ktrans(s):
                g, ls = divmod(s, 4)
                kt = kts[g]
                kp = psKT.tile([GP, 256], BF16, name=f"ktr_{s}", tag="ktr")
                nc.tensor.transpose(kp[:, 0:128], kt[:, ls * GP:ls * GP + GP], ident_sb)
                nc.tensor.transpose(kp[:, 128:256], kt[:, GW + ls * GP:GW + ls * GP + GP],
                                    ident_sb)
                k_sb = spool.tile([GP, 256], BF16, name=f"ksb_{s}", tag="ksb")
                nc.scalar.activation(k_sb, kp, COPY)
                return k_sb

            def masks(s, pt):
                comb = spool.tile([GP, GP], BF16, name=f"comb_{s}", tag="comb")
                tmp = spool.tile([GP, GP], BF16, name=f"tmp_{s}", tag="tmp")
                nc.vector.tensor_mul(comb, pt[:, 5:130], wct_sb)
                nc.vector.tensor_mul(tmp, pt[:, 0:GP], wit_sb)
                nc.vector.tensor_add(comb, comb, tmp)
                return comb

            def tail():
                # chunk 799 (positions 3995:4000), intra-only
                xv7 = xg[7][2]
                v5ps = psV.tile([5, 256], F32, name="v5", tag="v")
                for h in (0, 1):
                    mm(v5ps, xv7[:, h, 500:GW], w_sb[:, h, 512:768],
                       start=(h == 0), stop=(h == 1))
                v5_sb = spool.tile([5, 256], BF16, name="v5sb", tag="v5sb")
                nc.vector.tensor_copy(v5_sb, v5ps)
                pt5 = psPT.tile([5, 5], F32, name="pt5", tag="pt")
                for h, off in ((0, 0), (1, GW)):
                    mm(pt5, kts[7][:, off + 500:off + GW],
                       qts[7][:, off + 500:off + GW], start=(h == 0), stop=(h == 1))
                c5 = spool.tile([5, 5], BF16, name="c5", tag="c5")
                nc.vector.tensor_mul(c5, pt5, blob_sb[0:5, C_WIT:C_WIT + 5])
                wtf = psW.tile([5, 256], F32, name="wtf", tag="wt")
                mm(wtf, c5, v5_sb, start=True, stop=True)
                wallf = spool.tile([5, 256], F32, name="wallf", tag="wallf")
                nc.scalar.activation(wallf, wtf, COPY)
                nc.sync.dma_start(out=out[SEQ - 5:SEQ], in_=wallf)

            # --- prologue: group 0/1 loads, group-0 projections, prep(0) ---
            load_group_x(0)
            load_group_x(1)
            proj_qkt(0, "qt")
            proj_qkt(0, "kt")
            v_cur = vproj(0)
            pt0 = ptmm(0)
            k_cur = ktrans(0)
            comb_cur = masks(0, pt0)
            ut_prev = None

            for s in range(NSC):
                g, ls = divmod(s, 4)
                nxt = s + 1 < NSC
                v_next = vproj(s + 1) if nxt else None

                wt = psW.tile([GP, 256], F32, name=f"wt_{s}", tag="wt")
                mm(wt, comb_cur, v_cur, start=True, stop=(s == 0))
                if s > 0:
                    qt = qts[g]
                    mm(wt, qt[:, ls * GP + 5:ls * GP + 130], ut_prev[:, 0:256],
                       start=False, stop=False)
                    mm(wt, qt[:, GW + ls * GP + 5:GW + ls * GP + 130],
                       ut_prev[:, 256:512], start=False, stop=True)

                pt = ptmm(s + 1) if nxt else None

                mm(u_ps[:, 0:256], k_cur[:, 0:128], v_cur,
                   start=False, stop=True, skip_group_check=True)
                mm(u_ps[:, 256:512], k_cur[:, 128:256], v_cur,
                   start=False, stop=True, skip_group_check=True)
                if nxt:
                    ut_prev = spool.tile([128, 512], BF16, name=f"ut_{s}", tag="ut")
                    nc.scalar.activation(ut_prev, u_ps, COPY)

                if ls == 1 and g + 1 < NG:
                    proj_qkt(g + 1, "qt")
                if ls == 2 and g + 1 < NG:
                    proj_qkt(g + 1, "kt")
                    if g + 2 < NG:
                        load_group_x(g + 2)

                if nxt:
                    k_next = ktrans(s + 1)
                    comb_next = masks(s + 1, pt)
                if s == 27:
                    tail()

                wall = spool.tile([GP, 256], F32, name=f"wall_{s}", tag="wall",
                                  bufs=3)
                nc.vector.tensor_copy(wall, wt)
                if s == 0:
                    nc.sync.dma_start(out=out[0:GP - 5], in_=wall[5:GP])
                else:
                    nc.sync.dma_start(out=out[s * GP - 5:s * GP + 120], in_=wall)
                if nxt:
                    v_cur, k_cur, comb_cur = v_next, k_next, comb_next

    return nc


def _col_scales():
    j = np.arange(SEQ) // B          # global chunk index
    sq = (np.float64(g6) ** j).astype(np.float32)
    sk = (np.float64(g6) ** (-j)).astype(np.float32)
    return sq, sk


def prep_core_inputs(xq2d, xk2d, xv2d, wqkv):
    sq, sk = _col_scales()
    return {
        "xqT": (xq2d.T * sq[None, :]).astype(ml_dtypes.bfloat16),
        "xkT": (xk2d.T * sk[None, :]).astype(ml_dtypes.bfloat16),
        "xvT": np.ascontiguousarray(xv2d.T).astype(ml_dtypes.bfloat16),
        "wqkv": wqkv.astype(ml_dtypes.bfloat16),
    }


def make_in_maps(inputs):
    """inputs: dict from setup_inputs (full batch). Returns per-core in_maps."""
    xq, xk, xv = inputs["xq"], inputs["xk"], inputs["xv"]
    wqkv = np.ascontiguousarray(np.concatenate(
        [np.asarray(inputs["Wq"], dtype=np.float32),
         np.asarray(inputs["Wk"], dtype=np.float32),
         np.asarray(inputs["Wv"], dtype=np.float32)], axis=1))
    in_maps = []
    for b in range(8):
        in_maps.append(prep_core_inputs(
            np.asarray(xq[b], dtype=np.float32),
            np.asarray(xk[b], dtype=np.float32),
            np.asarray(xv[b], dtype=np.float32), wqkv))
    return in_maps


_NC_CACHE = {}


def _get_nc():
    if "nc" not in _NC_CACHE:
        from concourse import bacc
        nc = bacc.Bacc("TRN2", target_bir_lowering=False, debug=False)
        build_kernel(nc)
        nc.compile()
        _NC_CACHE["nc"] = nc
    return _NC_CACHE["nc"]


def run(inputs, trace=False, **kwargs):
    """Run on 8 NeuronCores; returns (output [8,4000,256], BassKernelResults)."""
    from concourse.bass_utils import run_bass_kernel_spmd

    nc = _get_nc()
    in_maps = make_in_maps(inputs)
    res = run_bass_kernel_spmd(nc, in_maps, core_ids=list(range(8)),
                               trace=trace, **kwargs)
    out = np.stack([r["out"] for r in res.results], axis=0)
    return out, res


def kernel(**inputs) -> np.ndarray:
    out, _ = run(inputs)
    return out


# revision 5
# speedup vs baseline: 1.1312x; 1.0648x over previous
"""Bass/Tile kernel for chunkwise retention (nn_ChunkwiseRetention).

Shifted-window scheme (v2), per core = one batch element, seq 4000, B=5:

Windows of 125 positions shifted by -5: window s covers output positions
[s*125-5, s*125+120), and the V/K contraction range is the SAME shifted
span, so the seam (intra of the chunk straddling the superchunk boundary)
folds into the single combined masked matmul — no separate seam matmul.
The carry boundary moves one chunk earlier: carry_s = Q[s*125:+125] @
U_shift(s-1) with U_shift accumulating K^T V over shifted windows.

Host pre-scales xqT columns by g6^j and xkT by g6^-j (j = chunk index),
folding all cross-chunk decay into the projections (cross mask is 0/1).
All inputs and SBUF operands are bf16 (PE: 1 cycle/row at any moving
width, halved DMA); PSUM accumulation stays f32.

Per iteration s: V proj (s+1, shifted window), window matmuls for s
(comb + carry into one PSUM group), P~^T (s+1) = K~^T_slice x Q~^T_slice
(N=130: q cols shifted -5..+125), state update, group-ahead Q^T/K^T
projections (N=505), K pos-major via PE transpose, DVE mask build
(comb = wct*P[:,5:130] + wit*P[:,0:125]), Pool PSUM->SBUF copies.
Tail (chunk 799, intra-only) is issued early at s==27.

PSUM banks (8): qkt 2 + v 1 + pt 1 + ktr 1 + wt 2 + u 1.
"""
import numpy as np
import ml_dtypes

import concourse.bass as bass
import concourse.mybir as mybir
import concourse.tile as tile

GAMMA = 0.9865
B = 5
SEQ = 4000
FEAT = 256
DIM = 256
GP = 125              # window size (25 chunks)
NSC = SEQ // GP       # 32
NG = 8                # groups of 4 windows
GW = 505              # group buffer width (500 + 5 shift overlap)
F32 = mybir.dt.float32
F32R = mybir.dt.float32r
BF16 = mybir.dt.bfloat16
g6 = float(np.float64(GAMMA) ** 6)
COPY = mybir.ActivationFunctionType.Copy

# const blob column layout (f32)
C_WIT = 0             # [0:125)   intra mask, shifted coords
C_WCT = 125           # [125:250) 0/1 cross mask, shifted coords
C_Z = 250             # [250:762) zeros (row 0: zero matmul operands)
C_END = 762


def make_const_blob():
    j = np.arange(GP)
    jj, rr = j[:, None], j[None, :]
    witn = np.where((jj // B == rr // B) & (rr % B >= jj % B),
                    np.float64(GAMMA) ** (jj % B - rr % B), 0.0)
    wctn = (jj // B <= rr // B).astype(np.float64)
    blob = np.zeros((128, C_END), np.float32)
    blob[0:GP, C_WIT:C_WIT + GP] = witn.astype(np.float32)
    blob[0:GP, C_WCT:C_WCT + GP] = wctn.astype(np.float32)
    return blob


def build_kernel(nc: bass.Bass):
    xqT = nc.dram_tensor("xqT", [FEAT, SEQ], BF16, kind="ExternalInput").ap()
    xkT = nc.dram_tensor("xkT", [FEAT, SEQ], BF16, kind="ExternalInput").ap()
    xvT = nc.dram_tensor("xvT", [FEAT, SEQ], BF16, kind="ExternalInput").ap()
    wqkv = nc.dram_tensor("wqkv", [FEAT, 3 * DIM], BF16, kind="ExternalInput").ap()
    out = nc.dram_tensor("out", [SEQ, DIM], F32, kind="ExternalOutput").ap()

    blob_np = make_const_blob()
    ident_np = np.eye(128, dtype=ml_dtypes.bfloat16)
    mm = nc.tensor.matmul

    with tile.TileContext(nc) as tc:
        with (
            tc.tile_pool(name="consts", bufs=1) as cpool,
            tc.tile_pool(name="xin", bufs=3) as xpool,
            tc.tile_pool(name="qkt", bufs=2) as qpool,
            tc.tile_pool(name="work", bufs=2) as spool,
            tc.tile_pool(name="psT", bufs=1, space="PSUM") as psT,
            tc.tile_pool(name="psV", bufs=1, space="PSUM") as psV,
            tc.tile_pool(name="psPT", bufs=1, space="PSUM") as psPT,
            tc.tile_pool(name="psKT", bufs=1, space="PSUM") as psKT,
            tc.tile_pool(name="psW", bufs=2, space="PSUM") as psW,
            tc.tile_pool(name="psU", bufs=1, space="PSUM") as psU,
        ):
            # --- constants: blob (f32), bf16 identity, packed weights ---
            blob_sb = cpool.tile([128, C_END], F32, name="blob_sb")
            nc.sync.dma_start(out=blob_sb, in_=nc.inline_tensor(blob_np, "cblob").ap())
            ident_sb = cpool.tile([128, 128], BF16, name="ident_sb")
            nc.sync.dma_start(out=ident_sb, in_=nc.inline_tensor(ident_np, "cident").ap())
            wit_sb = blob_sb[0:GP, C_WIT:C_WIT + GP]
            wct_sb = blob_sb[0:GP, C_WCT:C_WCT + GP]
            ww_sb = blob_sb[0:GP, 0:2 * GP].rearrange("p (b c) -> p b c", b=2)
            w_sb = cpool.tile_from(wqkv.rearrange("(h p) d -> p h d", p=128))

            u_ps = psU.tile([128, 512], F32, name="u_state")

            # preamble: absorb const/weight DMA waits; zero-matmul sets the
            # U bank's data + has_written bits so state matmuls accumulate.
            mm(u_ps[0:1, 0:1], w_sb[:, 0, 0:1], w_sb[:, 0, 0:1],
               start=True, stop=True, skip_group_check=True)
            scr = spool.tile([1, 1], F32, name="scr", tag="scr")
            nc.vector.tensor_copy(scr, blob_sb[0:1, 0:1])
            scr2 = spool.tile([1, 1], BF16, name="scr2", tag="scr2")
            nc.scalar.activation(scr2, ident_sb[0:1, 0:1], COPY)
            mm(u_ps, blob_sb[0:1, C_Z:C_Z + 128].bitcast(F32R),
               blob_sb[0:1, C_Z:C_Z + 512].bitcast(F32R),
               start=True, stop=True, skip_group_check=True)

            xg = {}
            qts = {}
            kts = {}

            def load_group_x(g):
                tiles = []
                for nm, src in (("xq", xqT), ("xk", xkT), ("xv", xvT)):
                    t = xpool.tile([128, 2, GW], BF16, name=f"{nm}_{g}", tag=nm)
                    if g == 0:
                        nc.vector.memset(t[:, :, 0:5], 0.0)
                        nc.sync.dma_start(
                            out=t[:, :, 5:GW],
                            in_=src[:, 0:500].rearrange("(h p) a -> p h a", p=128))
                    else:
                        nc.sync.dma_start(
                            out=t,
                            in_=src[:, g * 500 - 5:g * 500 + 500]
                                .rearrange("(h p) a -> p h a", p=128))
                    tiles.append(t)
                xg[g] = tiles

            def proj_qkt(g, which):
                x = xg[g][0 if which == "qt" else 1]
                dlos = (0, 128) if which == "qt" else (256, 384)
                ps = psT.tile([128, 1024], F32, name=f"ps_{which}_{g}", tag="qkt")
                for off, dlo in ((0, dlos[0]), (512, dlos[1])):
                    for h in (0, 1):
                        mm(ps[:, off:off + GW], w_sb[:, h, dlo:dlo + 128],
                           x[:, h, :], start=(h == 0), stop=(h == 1))
                t = qpool.tile([128, 2, GW], BF16, name=f"{which}_{g}", tag=which)
                ps_v = ps.rearrange("p (b c) -> p b c", b=2)[:, :, 0:GW]
                nc.scalar.activation(t, ps_v, COPY)
                (qts if which == "qt" else kts)[g] = t

            def vproj(s):
                g, ls = divmod(s, 4)
                xv = xg[g][2]
                vs = psV.tile([GP, 256], F32, name=f"v_{s}", tag="v")
                for h in (0, 1):
                    mm(vs, xv[:, h, ls * GP:ls * GP + GP], w_sb[:, h, 512:768],
                       start=(h == 0), stop=(h == 1))
                v_sb = spool.tile([GP, 256], BF16, name=f"vsb_{s}", tag="vsb")
                nc.vector.tensor_copy(v_sb, vs)
                return v_sb

            def ptmm(s):
                g, ls = divmod(s, 4)
                qt, kt = qts[g], kts[g]
                pt = psPT.tile([GP, 130], F32, name=f"pt_{s}", tag="pt")
                for h in (0, 1):
                    mm(pt, kt[:, h, ls * GP:ls * GP + GP],
                       qt[:, h, ls * GP:ls * GP + 130],
                       start=(h == 0), stop=(h == 1))
                return pt

            def ktrans(s):
                g, ls = divmod(s, 4)
                kt = kts[g]
                kp = psKT.tile([GP, 256], BF16, name=f"ktr_{s}", tag="ktr")
                nc.tensor.transpose(kp[:, 0:128], kt[:, 0, ls * GP:ls * GP + GP],
                                    ident_sb)
                nc.tensor.transpose(kp[:, 128:256], kt[:, 1, ls * GP:ls * GP + GP],
                                    ident_sb)
                k_sb = spool.tile([GP, 256], BF16, name=f"ksb_{s}", tag="ksb")
                if s % 2 == 0:
                    nc.vector.tensor_copy(k_sb, kp)
                else:
                    nc.scalar.activation(k_sb, kp, COPY)
                return k_sb

            def masks(s, pt):
                # one DVE mul: in0 = pt at col offsets {0, 5} (2-block AP),
                # in1 = [wit | wct] blob block, out = both products; then the
                # (SBUF-only) add runs on the otherwise-idle Pool engine.
                c2 = spool.tile([GP, 2, GP], BF16, name=f"c2_{s}", tag="c2")
                pt_blocks = bass.AP(pt.tensor, pt.offset, [[130, GP], [5, 2], [1, GP]])
                nc.vector.tensor_mul(c2, pt_blocks, ww_sb)
                comb = spool.tile([GP, GP], BF16, name=f"comb_{s}", tag="comb")
                nc.gpsimd.tensor_add(comb, c2[:, 0, :], c2[:, 1, :])
                return comb

            def tail():
                # chunk 799 (positions 3995:4000), intra-only
                xv7 = xg[7][2]
                v5ps = psV.tile([5, 256], F32, name="v5", tag="v")
                for h in (0, 1):
                    mm(v5ps, xv7[:, h, 500:GW], w_sb[:, h, 512:768],
                       start=(h == 0), stop=(h == 1))
                v5_sb = spool.tile([5, 256], BF16, name="v5sb", tag="v5sb")
                nc.vector.tensor_copy(v5_sb, v5ps)
                pt5 = psPT.tile([5, 5], F32, name="pt5", tag="pt")
                for h in (0, 1):
                    mm(pt5, kts[7][:, h, 500:GW],
                       qts[7][:, h, 500:GW], start=(h == 0), stop=(h == 1))
                c5 = spool.tile([5, 5], BF16, name="c5", tag="c5")
                nc.vector.tensor_mul(c5, pt5, blob_sb[0:5, C_WIT:C_WIT + 5])
                wtf = psW.tile([5, 256], F32, name="wtf", tag="wt")
                mm(wtf, c5, v5_sb, start=True, stop=True)
                wallf = spool.tile([5, 256], F32, name="wallf", tag="wallf")
                nc.scalar.activation(wallf, wtf, COPY)
                nc.sync.dma_start(out=out[SEQ - 5:SEQ], in_=wallf)

            # --- prologue: group 0/1 loads, group-0 projections, prep(0) ---
            load_group_x(0)
            load_group_x(1)
            proj_qkt(0, "qt")
            proj_qkt(0, "kt")
            v_cur = vproj(0)
            pt0 = ptmm(0)
            k_cur = ktrans(0)
            comb_cur = masks(0, pt0)
            ut_prev = None
            wall_pair = {}

            for s in range(NSC):
                g, ls = divmod(s, 4)
                nxt = s + 1 < NSC
                v_next = vproj(s + 1) if nxt else None

                wt = psW.tile([GP, 256], F32, name=f"wt_{s}", tag="wt")
                mm(wt, comb_cur, v_cur, start=True, stop=(s == 0))
                if s > 0:
                    qt = qts[g]
                    mm(wt, qt[:, 0, ls * GP + 5:ls * GP + 130], ut_prev[:, 0:256],
                       start=False, stop=False)
                    mm(wt, qt[:, 1, ls * GP + 5:ls * GP + 130],
                       ut_prev[:, 256:512], start=False, stop=True)

                pt = ptmm(s + 1) if nxt else None

                mm(u_ps[:, 0:256], k_cur[:, 0:128], v_cur,
                   start=False, stop=True, skip_group_check=True)
                mm(u_ps[:, 256:512], k_cur[:, 128:256], v_cur,
                   start=False, stop=True, skip_group_check=True)
                if nxt:
                    ut_prev = spool.tile([128, 512], BF16, name=f"ut_{s}", tag="ut")
                    nc.scalar.activation(ut_prev, u_ps, COPY)

                if ls == 1 and g + 1 < NG:
                    proj_qkt(g + 1, "qt")
                if ls == 2 and g + 1 < NG:
                    proj_qkt(g + 1, "kt")
                    if g + 2 < NG:
                        load_group_x(g + 2)

                if nxt:
                    k_next = ktrans(s + 1)
                    comb_next = masks(s + 1, pt)
                if s == 27:
                    tail()

                if s < 2:
                    wall = spool.tile([GP, 256], F32, name=f"wall_{s}",
                                      tag="wall0", bufs=2)
                    nc.vector.tensor_copy(wall, wt)
                    if s == 0:
                        nc.sync.dma_start(out=out[0:GP - 5], in_=wall[5:GP])
                    else:
                        nc.sync.dma_start(out=out[s * GP - 5:s * GP + 120],
                                          in_=wall)
                else:
                    if s % 2 == 0:
                        wall2 = spool.tile([GP, 2, 256], F32, name=f"wall_{s}",
                                           tag="wall", bufs=2)
                        wall_pair[0] = wall2
                        nc.vector.tensor_copy(wall2[:, 0, :], wt)
                    else:
                        wall2 = wall_pair[0]
                        nc.vector.tensor_copy(wall2[:, 1, :], wt)
                        nc.sync.dma_start(
                            out=out[(s - 1) * GP - 5:(s + 1) * GP - 5]
                                .rearrange("(b p) d -> p b d", b=2),
                            in_=wall2)
                if nxt:
                    v_cur, k_cur, comb_cur = v_next, k_next, comb_next

    return nc


def _col_scales():
    j = np.arange(SEQ) // B          # global chunk index
    sq = (np.float64(g6) ** j).astype(np.float32)
    sk = (np.float64(g6) ** (-j)).astype(np.float32)
    return sq, sk


def prep_core_inputs(xq2d, xk2d, xv2d, wqkv):
    sq, sk = _col_scales()
    return {
        "xqT": (xq2d.T * sq[None, :]).astype(ml_dtypes.bfloat16),
        "xkT": (xk2d.T * sk[None, :]).astype(ml_dtypes.bfloat16),
        "xvT": np.ascontiguousarray(xv2d.T).astype(ml_dtypes.bfloat16),
        "wqkv": wqkv.astype(ml_dtypes.bfloat16),
    }


def make_in_maps(inputs):
    """inputs: dict from setup_inputs (full batch). Returns per-core in_maps."""
    xq, xk, xv = inputs["xq"], inputs["xk"], inputs["xv"]
    wqkv = np.ascontiguousarray(np.concatenate(
        [np.asarray(inputs["Wq"], dtype=np.float32),
         np.asarray(inputs["Wk"], dtype=np.float32),
         np.asarray(inputs["Wv"], dtype=np.float32)], axis=1))
    in_maps = []
    for b in range(8):
        in_maps.append(prep_core_inputs(
            np.asarray(xq[b], dtype=np.float32),
            np.asarray(xk[b], dtype=np.float32),
            np.asarray(xv[b], dtype=np.float32), wqkv))
    return in_maps


_NC_CACHE = {}


def _get_nc():
    if "nc" not in _NC_CACHE:
        from concourse import bacc
        nc = bacc.Bacc("TRN2", target_bir_lowering=False, debug=False)
        build_kernel(nc)
        nc.compile()
        _NC_CACHE["nc"] = nc
    return _NC_CACHE["nc"]


def run(inputs, trace=False, **kwargs):
    """Run on 8 NeuronCores; returns (output [8,4000,256], BassKernelResults)."""
    from concourse.bass_utils import run_bass_kernel_spmd

    nc = _get_nc()
    in_maps = make_in_maps(inputs)
    res = run_bass_kernel_spmd(nc, in_maps, core_ids=list(range(8)),
                               trace=trace, **kwargs)
    out = np.stack([r["out"] for r in res.results], axis=0)
    return out, res


def kernel(**inputs) -> np.ndarray:
    out, _ = run(inputs)
    return out


# revision 6
# speedup vs baseline: 1.1545x; 1.0206x over previous
"""Bass/Tile kernel for chunkwise retention (nn_ChunkwiseRetention).

Shifted-window scheme (v2), per core = one batch element, seq 4000, B=5:

Windows of 125 positions shifted by -5: window s covers output positions
[s*125-5, s*125+120), and the V/K contraction range is the SAME shifted
span, so the seam (intra of the chunk straddling the superchunk boundary)
folds into the single combined masked matmul — no separate seam matmul.
The carry boundary moves one chunk earlier: carry_s = Q[s*125:+125] @
U_shift(s-1) with U_shift accumulating K^T V over shifted windows.

Host pre-scales xqT columns by g6^j and xkT by g6^-j (j = chunk index),
folding all cross-chunk decay into the projections (cross mask is 0/1).
All inputs and SBUF operands are bf16 (PE: 1 cycle/row at any moving
width, halved DMA); PSUM accumulation stays f32.

Per iteration s: V proj (s+1, shifted window), window matmuls for s
(comb + carry into one PSUM group), P~^T (s+1) at N=130 (q cols shifted
-5..+125), state update, group-ahead Q^T/K^T projections (N=505, fused
single PSUM->SBUF copy), K pos-major via PE transposes into a bf16
bitcast region of the same PSUM tile as P~^T (shared bank, bufs=2 so
the mask chain has a full iteration of slack), one DVE mask mul via a
2-block strided AP + SBUF-only add on the Pool engine, paired output
DMAs. Tail (chunk 799, intra-only) is issued early at s==27.

PSUM banks (8): qkt 2 + v 1 + (pt|ktr) 2 + wt 2 + u 1.
"""
import numpy as np
import ml_dtypes

import concourse.bass as bass
import concourse.mybir as mybir
import concourse.tile as tile

GAMMA = 0.9865
B = 5
SEQ = 4000
FEAT = 256
DIM = 256
GP = 125              # window size (25 chunks)
NSC = SEQ // GP       # 32
NG = 8                # groups of 4 windows
GW = 505              # group buffer width (500 + 5 shift overlap)
F32 = mybir.dt.float32
F32R = mybir.dt.float32r
BF16 = mybir.dt.bfloat16
g6 = float(np.float64(GAMMA) ** 6)
COPY = mybir.ActivationFunctionType.Copy

# const blob column layout (f32)
C_WIT = 0             # [0:125)   intra mask, shifted coords
C_WCT = 125           # [125:250) 0/1 cross mask, shifted coords
C_Z = 250             # [250:762) zeros (row 0: zero matmul operands)
C_END = 762


def make_const_blob():
    j = np.arange(GP)
    jj, rr = j[:, None], j[None, :]
    witn = np.where((jj // B == rr // B) & (rr % B >= jj % B),
                    np.float64(GAMMA) ** (jj % B - rr % B), 0.0)
    wctn = (jj // B <= rr // B).astype(np.float64)
    blob = np.zeros((128, C_END), np.float32)
    blob[0:GP, C_WIT:C_WIT + GP] = witn.astype(np.float32)
    blob[0:GP, C_WCT:C_WCT + GP] = wctn.astype(np.float32)
    return blob


def build_kernel(nc: bass.Bass):
    xqT = nc.dram_tensor("xqT", [FEAT, SEQ], BF16, kind="ExternalInput").ap()
    xkT = nc.dram_tensor("xkT", [FEAT, SEQ], BF16, kind="ExternalInput").ap()
    xvT = nc.dram_tensor("xvT", [FEAT, SEQ], BF16, kind="ExternalInput").ap()
    wqkv = nc.dram_tensor("wqkv", [FEAT, 3 * DIM], BF16, kind="ExternalInput").ap()
    out = nc.dram_tensor("out", [SEQ, DIM], F32, kind="ExternalOutput").ap()

    blob_np = make_const_blob()
    ident_np = np.eye(128, dtype=ml_dtypes.bfloat16)
    mm = nc.tensor.matmul

    with tile.TileContext(nc) as tc:
        with (
            tc.tile_pool(name="consts", bufs=1) as cpool,
            tc.tile_pool(name="xin", bufs=3) as xpool,
            tc.tile_pool(name="qkt", bufs=2) as qpool,
            tc.tile_pool(name="work", bufs=2) as spool,
            tc.tile_pool(name="psT", bufs=1, space="PSUM") as psT,
            tc.tile_pool(name="psV", bufs=1, space="PSUM") as psV,
            tc.tile_pool(name="psX", bufs=2, space="PSUM") as psX,
            tc.tile_pool(name="psW", bufs=2, space="PSUM") as psW,
            tc.tile_pool(name="psU", bufs=1, space="PSUM") as psU,
        ):
            # weights DMA first (first projection gates on it), then group-0
            # x loads, then the const blob / identity, then group 1.
            w_sb = cpool.tile_from(wqkv.rearrange("(h p) d -> p h d", p=128))
            blob_sb = cpool.tile([128, C_END], F32, name="blob_sb")
            ident_sb = cpool.tile([128, 128], BF16, name="ident_sb")
            wit_sb = blob_sb[0:GP, C_WIT:C_WIT + GP]
            ww_sb = blob_sb[0:GP, 0:2 * GP].rearrange("p (b c) -> p b c", b=2)

            u_ps = psU.tile([128, 512], F32, name="u_state")

            xg = {}
            qts = {}
            kts = {}

            def load_group_x(g):
                tiles = []
                for nm, src in (("xq", xqT), ("xk", xkT), ("xv", xvT)):
                    t = xpool.tile([128, 2, GW], BF16, name=f"{nm}_{g}", tag=nm)
                    if g == 0:
                        nc.vector.memset(t[:, :, 0:5], 0.0)
                        nc.sync.dma_start(
                            out=t[:, :, 5:GW],
                            in_=src[:, 0:500].rearrange("(h p) a -> p h a", p=128))
                    else:
                        nc.sync.dma_start(
                            out=t,
                            in_=src[:, g * 500 - 5:g * 500 + 500]
                                .rearrange("(h p) a -> p h a", p=128))
                    tiles.append(t)
                xg[g] = tiles

            def proj_qkt(g, which):
                x = xg[g][0 if which == "qt" else 1]
                dlos = (0, 128) if which == "qt" else (256, 384)
                ps = psT.tile([128, 1024], F32, name=f"ps_{which}_{g}", tag="qkt")
                for off, dlo in ((0, dlos[0]), (512, dlos[1])):
                    for h in (0, 1):
                        mm(ps[:, off:off + GW], w_sb[:, h, dlo:dlo + 128],
                           x[:, h, :], start=(h == 0), stop=(h == 1))
                t = qpool.tile([128, 2, GW], BF16, name=f"{which}_{g}", tag=which)
                ps_v = ps.rearrange("p (b c) -> p b c", b=2)[:, :, 0:GW]
                nc.scalar.activation(t, ps_v, COPY)
                (qts if which == "qt" else kts)[g] = t

            def vproj(s):
                g, ls = divmod(s, 4)
                xv = xg[g][2]
                vs = psV.tile([GP, 256], F32, name=f"v_{s}", tag="v")
                for h in (0, 1):
                    mm(vs, xv[:, h, ls * GP:ls * GP + GP], w_sb[:, h, 512:768],
                       start=(h == 0), stop=(h == 1))
                v_sb = spool.tile([GP, 256], BF16, name=f"vsb_{s}", tag="vsb")
                nc.vector.tensor_copy(v_sb, vs)
                return v_sb

            def ptmm(s):
                # P~^T in cols 0:130 (f32) of a [125, 512] tile whose bytes
                # 1536:2048 also hold the K-transpose output (bf16 bitcast) —
                # one bank, two bufs, so the mask chain isn't serialized.
                g, ls = divmod(s, 4)
                qt, kt = qts[g], kts[g]
                px = psX.tile([GP, 512], F32, name=f"px_{s}", tag="px")
                for h in (0, 1):
                    mm(px[:, 0:130], kt[:, h, ls * GP:ls * GP + GP],
                       qt[:, h, ls * GP:ls * GP + 130],
                       start=(h == 0), stop=(h == 1))
                return px

            def ktrans(s, px):
                g, ls = divmod(s, 4)
                kt = kts[g]
                kv = px[:, 384:512].bitcast(BF16)   # [125, 256] bf16 region
                mm(kv[:, 0:128], kt[:, 0, ls * GP:ls * GP + GP], ident_sb,
                   is_transpose=True, skip_group_check=True)
                mm(kv[:, 128:256], kt[:, 1, ls * GP:ls * GP + GP], ident_sb,
                   is_transpose=True, skip_group_check=True)
                k_sb = spool.tile([GP, 256], BF16, name=f"ksb_{s}", tag="ksb")
                if s % 2 == 0:
                    nc.vector.tensor_copy(k_sb, kv)
                else:
                    nc.scalar.activation(k_sb, kv, COPY)
                return k_sb

            def masks(s, px):
                # one DVE mul: in0 = pt at col offsets {0, 5} (2-block AP),
                # in1 = [wit | wct] blob block, out = both products; the
                # SBUF-only add runs on the otherwise-idle Pool engine.
                c2 = spool.tile([GP, 2, GP], BF16, name=f"c2_{s}", tag="c2")
                pt_blocks = bass.AP(px.tensor, px.offset,
                                    [[512, GP], [5, 2], [1, GP]])
                nc.vector.tensor_mul(c2, pt_blocks, ww_sb)
                comb = spool.tile([GP, GP], BF16, name=f"comb_{s}", tag="comb")
                nc.gpsimd.tensor_add(comb, c2[:, 0, :], c2[:, 1, :])
                return comb

            def tail():
                # chunk 799 (positions 3995:4000), intra-only
                xv7 = xg[7][2]
                v5ps = psV.tile([5, 256], F32, name="v5", tag="v")
                for h in (0, 1):
                    mm(v5ps, xv7[:, h, 500:GW], w_sb[:, h, 512:768],
                       start=(h == 0), stop=(h == 1))
                v5_sb = spool.tile([5, 256], BF16, name="v5sb", tag="v5sb")
                nc.vector.tensor_copy(v5_sb, v5ps)
                px5 = psX.tile([GP, 512], F32, name="px5", tag="px")
                for h in (0, 1):
                    mm(px5[0:5, 0:5], kts[7][:, h, 500:GW],
                       qts[7][:, h, 500:GW], start=(h == 0), stop=(h == 1))
                c5 = spool.tile([5, 5], BF16, name="c5", tag="c5")
                nc.vector.tensor_mul(c5, px5[0:5, 0:5], blob_sb[0:5, C_WIT:C_WIT + 5])
                wtf = psW.tile([5, 256], F32, name="wtf", tag="wt")
                mm(wtf, c5, v5_sb, start=True, stop=True)
                wallf = spool.tile([5, 256], F32, name="wallf", tag="wallf")
                nc.scalar.activation(wallf, wtf, COPY)
                nc.sync.dma_start(out=out[SEQ - 5:SEQ], in_=wallf)

            # --- prologue ---
            load_group_x(0)
            nc.sync.dma_start(out=blob_sb, in_=nc.inline_tensor(blob_np, "cblob").ap())
            nc.sync.dma_start(out=ident_sb,
                              in_=nc.inline_tensor(ident_np, "cident").ap())
            load_group_x(1)

            # preamble: absorb const/weight DMA waits; zero-matmul sets the
            # U bank's data + has_written bits so state matmuls accumulate.
            mm(u_ps[0:1, 0:1], w_sb[:, 0, 0:1], w_sb[:, 0, 0:1],
               start=True, stop=True, skip_group_check=True)
            scr = spool.tile([1, 1], F32, name="scr", tag="scr")
            nc.vector.tensor_copy(scr, blob_sb[0:1, 0:1])
            scr2 = spool.tile([1, 1], BF16, name="scr2", tag="scr2")
            nc.scalar.activation(scr2, ident_sb[0:1, 0:1], COPY)
            mm(u_ps, blob_sb[0:1, C_Z:C_Z + 128].bitcast(F32R),
               blob_sb[0:1, C_Z:C_Z + 512].bitcast(F32R),
               start=True, stop=True, skip_group_check=True)

            proj_qkt(0, "qt")
            proj_qkt(0, "kt")
            v_cur = vproj(0)
            px0 = ptmm(0)
            k_cur = ktrans(0, px0)
            comb_cur = masks(0, px0)
            ut_prev = None
            wall_pair = {}

            for s in range(NSC):
                g, ls = divmod(s, 4)
                nxt = s + 1 < NSC
                v_next = vproj(s + 1) if nxt else None

                wt = psW.tile([GP, 256], F32, name=f"wt_{s}", tag="wt")
                mm(wt, comb_cur, v_cur, start=True, stop=(s == 0))
                if s > 0:
                    qt = qts[g]
                    mm(wt, qt[:, 0, ls * GP + 5:ls * GP + 130], ut_prev[:, 0:256],
                       start=False, stop=False)
                    mm(wt, qt[:, 1, ls * GP + 5:ls * GP + 130],
                       ut_prev[:, 256:512], start=False, stop=True)

                px = ptmm(s + 1) if nxt else None

                mm(u_ps[:, 0:256], k_cur[:, 0:128], v_cur,
                   start=False, stop=True, skip_group_check=True)
                mm(u_ps[:, 256:512], k_cur[:, 128:256], v_cur,
                   start=False, stop=True, skip_group_check=True)
                if nxt:
                    ut_prev = spool.tile([128, 512], BF16, name=f"ut_{s}", tag="ut")
                    nc.scalar.activation(ut_prev, u_ps, COPY)

                if ls == 1 and g + 1 < NG:
                    proj_qkt(g + 1, "qt")
                if ls == 2 and g + 1 < NG:
                    proj_qkt(g + 1, "kt")
                    if g + 2 < NG:
                        load_group_x(g + 2)

                if nxt:
                    k_next = ktrans(s + 1, px)
                    comb_next = masks(s + 1, px)
                if s == 27:
                    tail()

                if s < 2:
                    wall = spool.tile([GP, 256], F32, name=f"wall_{s}",
                                      tag="wall0", bufs=2)
                    nc.vector.tensor_copy(wall, wt)
                    if s == 0:
                        nc.sync.dma_start(out=out[0:GP - 5], in_=wall[5:GP])
                    else:
                        nc.sync.dma_start(out=out[s * GP - 5:s * GP + 120],
                                          in_=wall)
                else:
                    if s % 2 == 0:
                        wall2 = spool.tile([GP, 2, 256], F32, name=f"wall_{s}",
                                           tag="wall", bufs=2)
                        wall_pair[0] = wall2
                        nc.vector.tensor_copy(wall2[:, 0, :], wt)
                    else:
                        wall2 = wall_pair[0]
                        nc.vector.tensor_copy(wall2[:, 1, :], wt)
                        nc.sync.dma_start(
                            out=out[(s - 1) * GP - 5:(s + 1) * GP - 5]
                                .rearrange("(b p) d -> p b d", b=2),
                            in_=wall2)
                if nxt:
                    v_cur, k_cur, comb_cur = v_next, k_next, comb_next

    return nc


def _col_scales():
    j = np.arange(SEQ) // B          # global chunk index
    sq = (np.float64(g6) ** j).astype(np.float32)
    sk = (np.float64(g6) ** (-j)).astype(np.float32)
    return sq, sk


def prep_core_inputs(xq2d, xk2d, xv2d, wqkv):
    sq, sk = _col_scales()
    return {
        "xqT": (xq2d.T * sq[None, :]).astype(ml_dtypes.bfloat16),
        "xkT": (xk2d.T * sk[None, :]).astype(ml_dtypes.bfloat16),
        "xvT": np.ascontiguousarray(xv2d.T).astype(ml_dtypes.bfloat16),
        "wqkv": wqkv.astype(ml_dtypes.bfloat16),
    }


def make_in_maps(inputs):
    """inputs: dict from setup_inputs (full batch). Returns per-core in_maps."""
    xq, xk, xv = inputs["xq"], inputs["xk"], inputs["xv"]
    wqkv = np.ascontiguousarray(np.concatenate(
        [np.asarray(inputs["Wq"], dtype=np.float32),
         np.asarray(inputs["Wk"], dtype=np.float32),
         np.asarray(inputs["Wv"], dtype=np.float32)], axis=1))
    in_maps = []
    for b in range(8):
        in_maps.append(prep_core_inputs(
            np.asarray(xq[b], dtype=np.float32),
            np.asarray(xk[b], dtype=np.float32),
            np.asarray(xv[b], dtype=np.float32), wqkv))
    return in_maps


_NC_CACHE = {}


def _get_nc():
    if "nc" not in _NC_CACHE:
        from concourse import bacc
        nc = bacc.Bacc("TRN2", target_bir_lowering=False, debug=False)
        build_kernel(nc)
        nc.compile()
        _NC_CACHE["nc"] = nc
    return _NC_CACHE["nc"]


def run(inputs, trace=False, **kwargs):
    """Run on 8 NeuronCores; returns (output [8,4000,256], BassKernelResults)."""
    from concourse.bass_utils import run_bass_kernel_spmd

    nc = _get_nc()
    in_maps = make_in_maps(inputs)
    res = run_bass_kernel_spmd(nc, in_maps, core_ids=list(range(8)),
                               trace=trace, **kwargs)
    out = np.stack([r["out"] for r in res.results], axis=0)
    return out, res


def kernel(**inputs) -> np.ndarray:
    out, _ = run(inputs)
    return out


# revision 7
# speedup vs baseline: 1.2041x; 1.0429x over previous
"""Bass/Tile kernel for chunkwise retention (nn_ChunkwiseRetention).

Shifted-window scheme (v2), per core = one batch element, seq 4000, B=5:

Windows of 125 positions shifted by -5: window s covers output positions
[s*125-5, s*125+120), and the V/K contraction range is the SAME shifted
span, so the seam (intra of the chunk straddling the superchunk boundary)
folds into the single combined masked matmul — no separate seam matmul.
The carry boundary moves one chunk earlier: carry_s = Q[s*125:+125] @
U_shift(s-1) with U_shift accumulating K^T V over shifted windows.

Host pre-scales xqT columns by g6^j and xkT by g6^-j (j = chunk index),
folding all cross-chunk decay into the projections (cross mask is 0/1).
All inputs and SBUF operands are bf16 (PE: 1 cycle/row at any moving
width, halved DMA); PSUM accumulation stays f32.

Per iteration s: V proj (s+1, shifted window), window matmuls for s
(comb + carry into one PSUM group), P~^T (s+1) at N=130 (q cols shifted
-5..+125), state update, group-ahead Q^T/K^T projections (N=505, fused
single PSUM->SBUF copy), K pos-major via PE transposes into a bf16
bitcast region of the same PSUM tile as P~^T (shared bank, bufs=2 so
the mask chain has a full iteration of slack), one DVE mask mul via a
2-block strided AP + SBUF-only add on the Pool engine, paired output
DMAs. Tail (chunk 799, intra-only) is issued early at s==27.

PSUM banks (8): qkt 2 + v 1 + (pt|ktr) 2 + wt 2 + u 1.
"""
import numpy as np
import ml_dtypes

import concourse.bass as bass
import concourse.mybir as mybir
import concourse.tile as tile

GAMMA = 0.9865
B = 5
SEQ = 4000
FEAT = 256
DIM = 256
GP = 125              # window size (25 chunks)
NSC = SEQ // GP       # 32
NG = 8                # groups of 4 windows
GW = 505              # group buffer width (500 + 5 shift overlap)
F32 = mybir.dt.float32
F32R = mybir.dt.float32r
BF16 = mybir.dt.bfloat16
g6 = float(np.float64(GAMMA) ** 6)
COPY = mybir.ActivationFunctionType.Copy

# const blob column layout (f32)
C_WIT = 0             # [0:125)   intra mask, shifted coords
C_WCT = 125           # [125:250) 0/1 cross mask, shifted coords
C_Z = 250             # [250:762) zeros (row 0: zero matmul operands)
C_END = 762


def make_const_blob():
    j = np.arange(GP)
    jj, rr = j[:, None], j[None, :]
    witn = np.where((jj // B == rr // B) & (rr % B >= jj % B),
                    np.float64(GAMMA) ** (jj % B - rr % B), 0.0)
    wctn = (jj // B <= rr // B).astype(np.float64)
    blob = np.zeros((128, C_END), np.float32)
    blob[0:GP, C_WIT:C_WIT + GP] = witn.astype(np.float32)
    blob[0:GP, C_WCT:C_WCT + GP] = wctn.astype(np.float32)
    return blob


def build_kernel(nc: bass.Bass):
    xqT = nc.dram_tensor("xqT", [FEAT, SEQ], BF16, kind="ExternalInput").ap()
    xkT = nc.dram_tensor("xkT", [FEAT, SEQ], BF16, kind="ExternalInput").ap()
    xvT = nc.dram_tensor("xvT", [FEAT, SEQ], BF16, kind="ExternalInput").ap()
    wqkv = nc.dram_tensor("wqkv", [FEAT, 3 * DIM], BF16, kind="ExternalInput").ap()
    out = nc.dram_tensor("out", [SEQ, DIM], F32, kind="ExternalOutput").ap()

    blob_np = make_const_blob()
    ident_np = np.eye(128, dtype=ml_dtypes.bfloat16)
    mm = nc.tensor.matmul

    with tile.TileContext(nc) as tc:
        with (
            tc.tile_pool(name="consts", bufs=1) as cpool,
            tc.tile_pool(name="xin", bufs=3) as xpool,
            tc.tile_pool(name="qkt", bufs=2) as qpool,
            tc.tile_pool(name="work", bufs=2) as spool,
            tc.tile_pool(name="psT", bufs=1, space="PSUM") as psT,
            tc.tile_pool(name="psV", bufs=1, space="PSUM") as psV,
            tc.tile_pool(name="psX", bufs=2, space="PSUM") as psX,
            tc.tile_pool(name="psW", bufs=2, space="PSUM") as psW,
            tc.tile_pool(name="psU", bufs=1, space="PSUM") as psU,
        ):
            # weights DMA first (first projection gates on it), then group-0
            # x loads, then the const blob / identity, then group 1.
            w_sb = cpool.tile_from(wqkv.rearrange("(h p) d -> p h d", p=128))
            blob_sb = cpool.tile([128, C_END], F32, name="blob_sb")
            ident_sb = cpool.tile([128, 128], BF16, name="ident_sb")
            wit_sb = blob_sb[0:GP, C_WIT:C_WIT + GP]
            ww_sb = blob_sb[0:GP, 0:2 * GP].rearrange("p (b c) -> p b c", b=2)

            u_ps = psU.tile([128, 512], F32, name="u_state")

            xg = {}
            qts = {}
            kts = {}

            def load_group_x(g):
                tiles = []
                for nm, src in (("xq", xqT), ("xk", xkT), ("xv", xvT)):
                    t = xpool.tile([128, 2, GW], BF16, name=f"{nm}_{g}", tag=nm)
                    if g == 0:
                        nc.vector.memset(t[:, :, 0:5], 0.0)
                        nc.sync.dma_start(
                            out=t[:, :, 5:GW],
                            in_=src[:, 0:500].rearrange("(h p) a -> p h a", p=128))
                    else:
                        nc.sync.dma_start(
                            out=t,
                            in_=src[:, g * 500 - 5:g * 500 + 500]
                                .rearrange("(h p) a -> p h a", p=128))
                    tiles.append(t)
                xg[g] = tiles

            def proj_qkt(g, which):
                x = xg[g][0 if which == "qt" else 1]
                dlos = (0, 128) if which == "qt" else (256, 384)
                ps = psT.tile([128, 1024], F32, name=f"ps_{which}_{g}", tag="qkt")
                for off, dlo in ((0, dlos[0]), (512, dlos[1])):
                    for h in (0, 1):
                        mm(ps[:, off:off + GW], w_sb[:, h, dlo:dlo + 128],
                           x[:, h, :], start=(h == 0), stop=(h == 1))
                t = qpool.tile([128, 2, GW], BF16, name=f"{which}_{g}", tag=which)
                ps_v = ps.rearrange("p (b c) -> p b c", b=2)[:, :, 0:GW]
                nc.scalar.activation(t, ps_v, COPY)
                (qts if which == "qt" else kts)[g] = t

            def vproj(s):
                g, ls = divmod(s, 4)
                xv = xg[g][2]
                vs = psV.tile([GP, 256], F32, name=f"v_{s}", tag="v")
                for h in (0, 1):
                    mm(vs, xv[:, h, ls * GP:ls * GP + GP], w_sb[:, h, 512:768],
                       start=(h == 0), stop=(h == 1))
                v_sb = spool.tile([GP, 256], BF16, name=f"vsb_{s}", tag="vsb")
                nc.vector.tensor_copy(v_sb, vs)
                return v_sb

            def ptmm(s):
                # P~^T in cols 0:130 (f32) of a [125, 512] tile whose bytes
                # 1536:2048 also hold the K-transpose output (bf16 bitcast) —
                # one bank, two bufs, so the mask chain isn't serialized.
                g, ls = divmod(s, 4)
                qt, kt = qts[g], kts[g]
                px = psX.tile([GP, 512], F32, name=f"px_{s}", tag="px")
                for h in (0, 1):
                    mm(px[:, 0:130], kt[:, h, ls * GP:ls * GP + GP],
                       qt[:, h, ls * GP:ls * GP + 130],
                       start=(h == 0), stop=(h == 1))
                return px

            def ktrans(s, px):
                g, ls = divmod(s, 4)
                kt = kts[g]
                kv = px[:, 384:512].bitcast(BF16)   # [125, 256] bf16 region
                mm(kv[:, 0:128], kt[:, 0, ls * GP:ls * GP + GP], ident_sb,
                   is_transpose=True, skip_group_check=True)
                mm(kv[:, 128:256], kt[:, 1, ls * GP:ls * GP + GP], ident_sb,
                   is_transpose=True, skip_group_check=True)
                k_sb = spool.tile([GP, 256], BF16, name=f"ksb_{s}", tag="ksb",
                                  bufs=3)
                if s % 2 == 0:
                    nc.vector.tensor_copy(k_sb, kv)
                else:
                    nc.scalar.activation(k_sb, kv, COPY)
                return k_sb

            def masks(s, px):
                # one DVE mul: in0 = pt at col offsets {0, 5} (2-block AP),
                # in1 = [wit | wct] blob block, out = both products; the
                # SBUF-only add runs on the otherwise-idle Pool engine.
                c2 = spool.tile([GP, 2, GP], BF16, name=f"c2_{s}", tag="c2")
                pt_blocks = bass.AP(px.tensor, px.offset,
                                    [[512, GP], [5, 2], [1, GP]])
                nc.vector.tensor_mul(c2, pt_blocks, ww_sb)
                comb = spool.tile([GP, GP], BF16, name=f"comb_{s}", tag="comb",
                                  bufs=3)
                nc.gpsimd.tensor_add(comb, c2[:, 0, :], c2[:, 1, :])
                return comb

            def tail():
                # chunk 799 (positions 3995:4000), intra-only
                xv7 = xg[7][2]
                v5ps = psV.tile([5, 256], F32, name="v5", tag="v")
                for h in (0, 1):
                    mm(v5ps, xv7[:, h, 500:GW], w_sb[:, h, 512:768],
                       start=(h == 0), stop=(h == 1))
                v5_sb = spool.tile([5, 256], BF16, name="v5sb", tag="v5sb")
                nc.vector.tensor_copy(v5_sb, v5ps)
                px5 = psX.tile([GP, 512], F32, name="px5", tag="px")
                for h in (0, 1):
                    mm(px5[0:5, 0:5], kts[7][:, h, 500:GW],
                       qts[7][:, h, 500:GW], start=(h == 0), stop=(h == 1))
                c5 = spool.tile([5, 5], BF16, name="c5", tag="c5")
                nc.vector.tensor_mul(c5, px5[0:5, 0:5], blob_sb[0:5, C_WIT:C_WIT + 5])
                wtf = psW.tile([5, 256], F32, name="wtf", tag="wt")
                mm(wtf, c5, v5_sb, start=True, stop=True)
                wallf = spool.tile([5, 256], F32, name="wallf", tag="wallf")
                nc.scalar.activation(wallf, wtf, COPY)
                nc.sync.dma_start(out=out[SEQ - 5:SEQ], in_=wallf)

            # --- prologue ---
            load_group_x(0)
            nc.sync.dma_start(out=blob_sb, in_=nc.inline_tensor(blob_np, "cblob").ap())
            nc.sync.dma_start(out=ident_sb,
                              in_=nc.inline_tensor(ident_np, "cident").ap())
            load_group_x(1)

            # preamble: absorb const/weight DMA waits; zero-matmul sets the
            # U bank's data + has_written bits so state matmuls accumulate.
            mm(u_ps[0:1, 0:1], w_sb[:, 0, 0:1], w_sb[:, 0, 0:1],
               start=True, stop=True, skip_group_check=True)
            scr = spool.tile([1, 1], F32, name="scr", tag="scr")
            nc.vector.tensor_copy(scr, blob_sb[0:1, 0:1])
            scr2 = spool.tile([1, 1], BF16, name="scr2", tag="scr2")
            nc.scalar.activation(scr2, ident_sb[0:1, 0:1], COPY)
            mm(u_ps, blob_sb[0:1, C_Z:C_Z + 128].bitcast(F32R),
               blob_sb[0:1, C_Z:C_Z + 512].bitcast(F32R),
               start=True, stop=True, skip_group_check=True)

            proj_qkt(0, "qt")
            proj_qkt(0, "kt")
            vs = {0: vproj(0)}
            ks = {}
            combs = {}
            for t in (0, 1):
                pxt = ptmm(t)
                ks[t] = ktrans(t, pxt)
                combs[t] = masks(t, pxt)
            ut_prev = None
            wall_pair = {}

            for s in range(NSC):
                g, ls = divmod(s, 4)
                if s + 1 < NSC:
                    vs[s + 1] = vproj(s + 1)

                wt = psW.tile([GP, 256], F32, name=f"wt_{s}", tag="wt")
                mm(wt, combs[s], vs[s], start=True, stop=(s == 0))
                if s > 0:
                    qt = qts[g]
                    mm(wt, qt[:, 0, ls * GP + 5:ls * GP + 130], ut_prev[:, 0:256],
                       start=False, stop=False)
                    mm(wt, qt[:, 1, ls * GP + 5:ls * GP + 130],
                       ut_prev[:, 256:512], start=False, stop=True)

                px = ptmm(s + 2) if s + 2 < NSC else None

                mm(u_ps[:, 0:256], ks[s][:, 0:128], vs[s],
                   start=False, stop=True, skip_group_check=True)
                mm(u_ps[:, 256:512], ks[s][:, 128:256], vs[s],
                   start=False, stop=True, skip_group_check=True)
                if s + 1 < NSC:
                    ut_prev = spool.tile([128, 512], BF16, name=f"ut_{s}", tag="ut")
                    nc.scalar.activation(ut_prev, u_ps, COPY)

                if ls == 0 and g + 1 < NG:
                    proj_qkt(g + 1, "qt")
                if ls == 1 and g + 1 < NG:
                    proj_qkt(g + 1, "kt")
                    if g + 2 < NG:
                        load_group_x(g + 2)

                if px is not None:
                    ks[s + 2] = ktrans(s + 2, px)
                    combs[s + 2] = masks(s + 2, px)
                if s == 26:
                    tail()

                if s < 2:
                    wall = spool.tile([GP, 256], F32, name=f"wall_{s}",
                                      tag="wall0", bufs=2)
                    nc.vector.tensor_copy(wall, wt)
                    if s == 0:
                        nc.sync.dma_start(out=out[0:GP - 5], in_=wall[5:GP])
                    else:
                        nc.sync.dma_start(out=out[s * GP - 5:s * GP + 120],
                                          in_=wall)
                else:
                    if s % 2 == 0:
                        wall2 = spool.tile([GP, 2, 256], F32, name=f"wall_{s}",
                                           tag="wall", bufs=2)
                        wall_pair[0] = wall2
                        nc.vector.tensor_copy(wall2[:, 0, :], wt)
                    else:
                        wall2 = wall_pair[0]
                        nc.vector.tensor_copy(wall2[:, 1, :], wt)
                        nc.sync.dma_start(
                            out=out[(s - 1) * GP - 5:(s + 1) * GP - 5]
                                .rearrange("(b p) d -> p b d", b=2),
                            in_=wall2)
                vs.pop(s, None)
                ks.pop(s, None)
                combs.pop(s, None)

    return nc


def _col_scales():
    j = np.arange(SEQ) // B          # global chunk index
    sq = (np.float64(g6) ** j).astype(np.float32)
    sk = (np.float64(g6) ** (-j)).astype(np.float32)
    return sq, sk


def prep_core_inputs(xq2d, xk2d, xv2d, wqkv):
    sq, sk = _col_scales()
    return {
        "xqT": (xq2d.T * sq[None, :]).astype(ml_dtypes.bfloat16),
        "xkT": (xk2d.T * sk[None, :]).astype(ml_dtypes.bfloat16),
        "xvT": np.ascontiguousarray(xv2d.T).astype(ml_dtypes.bfloat16),
        "wqkv": wqkv.astype(ml_dtypes.bfloat16),
    }


def make_in_maps(inputs):
    """inputs: dict from setup_inputs (full batch). Returns per-core in_maps."""
    xq, xk, xv = inputs["xq"], inputs["xk"], inputs["xv"]
    wqkv = np.ascontiguousarray(np.concatenate(
        [np.asarray(inputs["Wq"], dtype=np.float32),
         np.asarray(inputs["Wk"], dtype=np.float32),
         np.asarray(inputs["Wv"], dtype=np.float32)], axis=1))
    in_maps = []
    for b in range(8):
        in_maps.append(prep_core_inputs(
            np.asarray(xq[b], dtype=np.float32),
            np.asarray(xk[b], dtype=np.float32),
            np.asarray(xv[b], dtype=np.float32), wqkv))
    return in_maps


_NC_CACHE = {}


def _get_nc():
    if "nc" not in _NC_CACHE:
        from concourse import bacc
        nc = bacc.Bacc("TRN2", target_bir_lowering=False, debug=False)
        build_kernel(nc)
        nc.compile()
        _NC_CACHE["nc"] = nc
    return _NC_CACHE["nc"]


def run(inputs, trace=False, **kwargs):
    """Run on 8 NeuronCores; returns (output [8,4000,256], BassKernelResults)."""
    from concourse.bass_utils import run_bass_kernel_spmd

    nc = _get_nc()
    in_maps = make_in_maps(inputs)
    res = run_bass_kernel_spmd(nc, in_maps, core_ids=list(range(8)),
                               trace=trace, **kwargs)
    out = np.stack([r["out"] for r in res.results], axis=0)
    return out, res


def kernel(**inputs) -> np.ndarray:
    out, _ = run(inputs)
    return out
